# revision 1
# baseline (speedup 1.0000x reference)
"""Trainium2 Bass kernel for nn_CliffordEngine (8-core data-parallel over batch).

Model: 4 passes of (grouped causal 3x3x3 conv over 32^3 grid -> tanh ->
8x8 cross-field mix -> alpha blend), then a sigmoid gate vs the original
state.  B=16, F=8 fields, 8 multivector components, G=32.

Mapping: each core takes 2 batch elements.  SBUF layout: partitions =
(b2, f8, c8) = 128; free dim = causally padded 34^3 volume (bf16).  The
conv is 27 offset-matmuls (block-diagonal 128x128 weights) accumulating
in PSUM; tanh on ScalarE; mix is one more matmul; the alpha blend +
fp32->bf16 store is a single DVE scalar_tensor_tensor.
"""

import numpy as np

import concourse.bacc as bacc
import concourse.tile as tile
from concourse import mybir
from concourse.bass_utils import run_bass_kernel_spmd

# Cl(3,0) Cayley table, transcribed from the reference model.
_TABLE = [
    [(0, 0, 1), (1, 1, 1), (2, 2, 1), (3, 3, 1), (4, 4, -1), (5, 5, -1), (6, 6, -1), (7, 7, -1)],
    [(1, 0, 1), (0, 1, 1), (4, 2, -1), (5, 3, -1), (2, 4, 1), (3, 5, 1), (7, 6, -1), (6, 7, -1)],
    [(2, 0, 1), (4, 1, 1), (0, 2, 1), (6, 3, -1), (1, 4, -1), (7, 5, 1), (3, 6, 1), (5, 7, 1)],
    [(3, 0, 1), (5, 1, 1), (6, 2, 1), (0, 3, 1), (7, 4, -1), (1, 5, -1), (2, 6, -1), (4, 7, -1)],
    [(4, 0, 1), (2, 1, 1), (1, 2, -1), (7, 3, 1), (0, 4, 1), (6, 5, -1), (5, 6, 1), (3, 7, 1)],
    [(5, 0, 1), (3, 1, 1), (7, 2, -1), (1, 3, -1), (6, 4, 1), (0, 5, 1), (4, 6, -1), (2, 7, -1)],
    [(6, 0, 1), (7, 1, 1), (3, 2, 1), (2, 3, -1), (5, 4, -1), (4, 5, 1), (0, 6, 1), (1, 7, 1)],
    [(7, 0, 1), (6, 1, 1), (5, 2, -1), (4, 3, 1), (3, 4, 1), (2, 5, -1), (1, 6, 1), (0, 7, 1)],
]

B, F, C, G = 16, 8, 8, 32
P_PASSES = 4
NCORES = 8
BL = B // NCORES            # local batch per core = 2
NPART = BL * F * C          # 128
GP = G + 2                  # padded extent 34
G3 = G * G * G
NTAPS = 27

F32 = mybir.dt.float32
BF16 = mybir.dt.bfloat16
NP_BF16 = mybir.dt.np(BF16)


def _softmax(x, axis):
    m = np.max(x, axis=axis, keepdims=True)
    e = np.exp(x - m)
    return e / np.sum(e, axis=axis, keepdims=True)


def _host_params(all_weights, all_biases, field_mix_logits, pass_alpha_logit,
                 gate_weight, gate_bias):
    """Precompute device weight tensors (runtime data, not baked into the NEFF)."""
    T = np.zeros((8, 8, 8), np.float32)
    for i, row in enumerate(_TABLE):
        for j, k, s in row:
            T[i, j, k] = s
    aw = np.asarray(all_weights, np.float32)          # [F, P, 27, 8]
    # W_eff[f,p,i,j,t] : out component i, in component j, tap t
    W = np.einsum('ijk,fpck->fpijc', T, aw)            # [F,P,8,8,27]
    mix = _softmax(np.asarray(field_mix_logits, np.float32), axis=2)  # [P,F,F] (g,f)
    alpha = 1.0 / (1.0 + np.exp(-np.asarray(pass_alpha_logit, np.float32)))  # [P]

    # conv lhsT: [k=(b,f,j), p, t, m=(b,f,i)] block-diagonal over (b, f)
    convw = np.zeros((BL, F, C, P_PASSES, NTAPS, BL, F, C), np.float32)
    A = np.transpose(W, (0, 3, 1, 4, 2))               # [f, j, p, t, i]
    for b in range(BL):
        for f in range(F):
            convw[b, f, :, :, :, b, f, :] = A[f]
    convw = convw.reshape(NPART, P_PASSES, NTAPS, NPART).astype(NP_BF16)

    # mix lhsT: [k=(b,f,ci), p, m=(b,g,co)] = (1-alpha_p) mix[p,g,f] delta_b delta_c
    mixw = np.zeros((BL, F, C, P_PASSES, BL, F, C), np.float32)
    for b in range(BL):
        for p in range(P_PASSES):
            M2 = (1.0 - alpha[p]) * mix[p].T           # [f, g]
            for ci in range(C):
                mixw[b, :, ci, p, b, :, ci] = M2
    mixw = mixw.reshape(NPART, P_PASSES, NPART).astype(NP_BF16)

    # per-partition scalars: cols 0-3 bias_p, 4-7 alpha_p, 8 = -gw, 9 = -gb
    pvec = np.zeros((BL, F, C, 12), np.float32)
    ab = np.asarray(all_biases, np.float32)            # [F, P, C]
    for p in range(P_PASSES):
        pvec[:, :, :, p] = ab[None, :, p, :]
        pvec[:, :, :, 4 + p] = alpha[p]
    pvec[:, :, :, 8] = -np.asarray(gate_weight, np.float32)[None]
    pvec[:, :, :, 9] = -np.asarray(gate_bias, np.float32)[None]
    pvec = pvec.reshape(NPART, 12)
    return convw, mixw, pvec


def build_nc(repeat_passes=1, active_taps=None):
    """Build the per-core Bass program.  repeat_passes>1 is a timing variant.

    active_taps: optional per-pass tuple of tap indices to emit (taps whose
    weights are identically zero can be skipped exactly).
    """
    if active_taps is None:
        active_taps = tuple(tuple(range(NTAPS)) for _ in range(P_PASSES))
    nc = bacc.Bacc("TRN2", target_bir_lowering=False, debug=False,
                   num_devices=NCORES)
    state_in = nc.dram_tensor("state_in", [NPART, G3], F32, kind="ExternalInput")
    convw_d = nc.dram_tensor("convw", [NPART, P_PASSES, NTAPS, NPART], BF16,
                             kind="ExternalInput")
    mixw_d = nc.dram_tensor("mixw", [NPART, P_PASSES, NPART], BF16,
                            kind="ExternalInput")
    pvec_d = nc.dram_tensor("pvec", [NPART, 12], F32, kind="ExternalInput")
    out_d = nc.dram_tensor("out", [NPART, G3], F32, kind="ExternalOutput")

    # Persistent padded state buffers (ping/pong), bf16.
    ping = nc.alloc_sbuf_tensor("ping", [NPART, GP, GP, GP], BF16)
    pong = nc.alloc_sbuf_tensor("pong", [NPART, GP, GP, GP], BF16)

    taps = [(kd, kh, kw) for kd in range(3) for kh in range(3) for kw in range(3)]

    with tile.TileContext(nc) as tc:
        with (
            tc.tile_pool(name="const", bufs=1) as constp,
            tc.tile_pool(name="convw", bufs=2) as convwp,
            tc.tile_pool(name="stage", bufs=3) as stagep,
            tc.tile_pool(name="ytile", bufs=3) as yp,
            tc.tile_pool(name="gtile", bufs=2) as gp_,
            tc.tile_pool(name="psum1", bufs=6, space="PSUM") as ps1p,
            tc.tile_pool(name="psum2", bufs=2, space="PSUM") as ps2p,
        ):
            nc.gpsimd.memset(ping[:], 0)
            nc.gpsimd.memset(pong[:], 0)

            mixw_sb = constp.tile([NPART, P_PASSES, NPART], BF16, tag="mixw")
            nc.sync.dma_start(mixw_sb[:], mixw_d[:])
            pvec_sb = constp.tile([NPART, 12], F32, tag="pvec")
            nc.sync.dma_start(pvec_sb[:], pvec_d[:])

            # initial load: f32 half-plane -> bf16 padded interior
            for cch in range(2 * G):
                z, h = cch // 2, cch % 2
                st = stagep.tile([NPART, 16, G], F32, tag="stage")
                nc.sync.dma_start(st[:], state_in[:, cch * 512:(cch + 1) * 512])
                nc.vector.tensor_copy(
                    out=ping[:, z + 2, 16 * h + 2:16 * h + 18, 2:GP], in_=st[:])

            cur, nxt = ping, pong
            for rp in range(repeat_passes):
                for p in range(P_PASSES):
                    convw_sb = convwp.tile([NPART, NTAPS, NPART], BF16, tag="convw")
                    nc.sync.dma_start(convw_sb[:], convw_d[:, p])
                    for z0 in range(G):
                        for h in range(2):
                            ps1 = ps1p.tile([NPART, 16, G], F32, space="PSUM",
                                            tag="ps1")
                            act = active_taps[p]
                            for ti, t in enumerate(act):
                                kd, kh, kw = taps[t]
                                nc.tensor.matmul(
                                    out=ps1[:],
                                    lhsT=convw_sb[:, t],
                                    rhs=cur[:, z0 + kd,
                                            16 * h + kh:16 * h + kh + 16,
                                            kw:kw + G],
                                    start=(ti == 0),
                                    stop=(ti == len(act) - 1),
                                )
                            y = yp.tile([NPART, 16, G], BF16, tag="y")
                            nc.scalar.activation(
                                out=y[:], in_=ps1[:],
                                func=mybir.ActivationFunctionType.Tanh,
                                bias=pvec_sb[:, p:p + 1], scale=1.0,
                            )
                            ps2 = ps2p.tile([NPART, 16, G], F32, space="PSUM",
                                            tag="ps2")
                            nc.tensor.matmul(out=ps2[:], lhsT=mixw_sb[:, p],
                                             rhs=y[:], start=True, stop=True)
                            # nxt = alpha * cur + ps2   (cast to bf16)
                            nc.vector.scalar_tensor_tensor(
                                out=nxt[:, z0 + 2, 16 * h + 2:16 * h + 18, 2:GP],
                                in0=cur[:, z0 + 2, 16 * h + 2:16 * h + 18, 2:GP],
                                scalar=pvec_sb[:, 4 + p:5 + p],
                                in1=ps2[:],
                                op0=mybir.AluOpType.mult,
                                op1=mybir.AluOpType.add,
                            )
                    cur, nxt = nxt, cur

            # gate: out = old + (1-g) * (x4 - old), 1-g = sigmoid(-(gw*old+gb))
            for cch in range(64):
                sl = slice(cch * 512, (cch + 1) * 512)
                z0, h = cch // 2, cch % 2
                old = gp_.tile([NPART, 16, G], F32, tag="old")
                nc.sync.dma_start(old[:], state_in[:, sl])
                hh = gp_.tile([NPART, 16, G], F32, tag="hh")
                nc.scalar.activation(
                    out=hh[:], in_=old[:],
                    func=mybir.ActivationFunctionType.Sigmoid,
                    bias=pvec_sb[:, 9:10], scale=pvec_sb[:, 8:9],
                )
                d = gp_.tile([NPART, 16, G], F32, tag="d")
                nc.vector.tensor_tensor(
                    out=d[:],
                    in0=cur[:, z0 + 2, 16 * h + 2:16 * h + 18, 2:GP],
                    in1=old[:], op=mybir.AluOpType.subtract,
                )
                nc.vector.tensor_tensor(out=d[:], in0=hh[:], in1=d[:],
                                        op=mybir.AluOpType.mult)
                o = gp_.tile([NPART, 16, G], F32, tag="o")
                nc.vector.tensor_tensor(out=o[:], in0=old[:], in1=d[:],
                                        op=mybir.AluOpType.add)
                nc.sync.dma_start(out_d[:, sl], o[:])

    nc.compile()
    return nc


_NC_CACHE = {}


def _get_nc(repeat_passes=1, active_taps=None):
    key = (repeat_passes, active_taps)
    if key not in _NC_CACHE:
        _NC_CACHE[key] = build_nc(repeat_passes, active_taps)
    return _NC_CACHE[key]


def make_in_maps(state, all_weights, all_biases, field_mix_logits,
                 pass_alpha_logit, gate_weight, gate_bias):
    convw, mixw, pvec = _host_params(all_weights, all_biases, field_mix_logits,
                                     pass_alpha_logit, gate_weight, gate_bias)
    state = np.ascontiguousarray(np.asarray(state, np.float32))
    in_maps = []
    for i in range(NCORES):
        shard = state[BL * i:BL * (i + 1)].reshape(NPART, G3)
        in_maps.append({
            "state_in": shard,
            "convw": convw,
            "mixw": mixw,
            "pvec": pvec,
        })
    return in_maps


def kernel(state, all_weights, all_biases, field_mix_logits,
           pass_alpha_logit, gate_weight, gate_bias):
    in_maps = make_in_maps(state, all_weights, all_biases, field_mix_logits,
                           pass_alpha_logit, gate_weight, gate_bias)
    # skip taps whose conv weights are identically zero (exact specialization;
    # always keep at least one tap per pass so the PSUM group is well-formed)
    convw = in_maps[0]["convw"]
    active = []
    for p in range(P_PASSES):
        nz = tuple(t for t in range(NTAPS)
                   if np.any(convw[:, p, t, :] != 0))
        active.append(nz if nz else (0,))
    nc = _get_nc(1, tuple(active))
    for attempt in range(5):
        try:
            res = run_bass_kernel_spmd(nc, in_maps, core_ids=list(range(NCORES)))
            break
        except Exception:  # transient device-recovery errors
            if attempt == 4:
                raise
            import time as _time
            _time.sleep(10.0 * (attempt + 1))
    outs = [res.results[i]["out"].reshape(BL, F, C, G, G, G)
            for i in range(NCORES)]
    return np.concatenate(outs, axis=0).astype(np.float32)

